# revision 17
# baseline (speedup 1.0000x reference)
"""Trainium2 Bass kernel for a diagonal SSM layer.

Computes, for u [4, 4096, 1024]:
    lam = sigmoid(log_lambda)                 # [256]
    Bu  = einsum('bsd,nd->bsn', u, B_w)       # [4, 4096, 256]
    h_t = lam * h_{t-1} + Bu_t                # scan over s
    y   = einsum('bsn,dn->bsd', hs, C_w) + D * u

Sharding: 8 cores = 4 batches x 2 sequence halves (2048 steps each).
Parameters are replicated. The half-boundary state is exchanged between
core pairs (2b, 2b+1) with a tiny AllGather; each core then re-runs its
(cheap) scan seeded with the inherited state. Uniform SPMD program —
even cores multiply the inherited state by a per-core flag input of 0.

Per-core dataflow:
  DMA u -> PE transpose (128x128 blocks, via identity matmul) -> uT
  BuT[n,t] = (B_w^T)^T @ uT          (float32r matmuls, K=1024)
  scan1 over t (DVE tensor_tensor_scan, state dim on partitions)
  exchange final state within the core pair; scan2 with inherited init
  y[t,:]  = hsT^T @ C_w^T (+ D*u)    (float32r matmuls, K=256)
  DMA y out

float32r is tf32 on the PE (10-bit mantissa, full-rate); everything
outside the matmuls stays fp32, overall relative error ~2e-4.

Two program variants are built: the D==0 fast path skips the D*u
elementwise stream (D is exactly zero for this layer's initialization);
any nonzero D dispatches to the general variant at call time.
"""

import sys

import numpy as np

sys.path.insert(0, "/opt/trn_rl_repo")

from concourse import bacc, mybir  # noqa: E402
import concourse.tile as tile  # noqa: E402
from concourse.bass_utils import run_bass_kernel_spmd  # noqa: E402

BATCH, SEQ, DM, SD = 4, 4096, 1024, 256
NCORES = 8
TH = SEQ // 2  # timesteps per core
NTC = TH // 512  # 512-step chunks per core
KD = DM // 128  # contraction chunks for the B matmul
NSC = SD // 128  # state chunks

F32 = mybir.dt.float32
F32R = mybir.dt.float32r

GROUPS = [[0, 1], [2, 3], [4, 5], [6, 7]]


def _r(ap):
    """float32r (tf32) view: full-rate fp32 matmul on the PE array."""
    return ap.bitcast(F32R)


def _f(ap):
    """plain-fp32 view of a float32r tile (bitwise identical)."""
    return ap.bitcast(F32)


def build_program(loop_n=1, with_du=False, num_devices=NCORES):
    nc = bacc.Bacc(
        "TRN2", target_bir_lowering=False, debug=False, num_devices=num_devices
    )

    u_d = nc.dram_tensor("u", [TH, DM], F32, kind="ExternalInput").ap()
    ll_d = nc.dram_tensor("logl", [SD], F32, kind="ExternalInput").ap()
    bt_d = nc.dram_tensor("bt", [DM, SD], F32R, kind="ExternalInput").ap()
    ct_d = nc.dram_tensor("ct", [SD, DM], F32R, kind="ExternalInput").ap()
    db_d = nc.dram_tensor("dbc", [128, DM], F32, kind="ExternalInput").ap()
    fl_d = nc.dram_tensor("flag", [128, 1], F32, kind="ExternalInput").ap()
    y_d = nc.dram_tensor("y", [TH, DM], F32, kind="ExternalOutput").ap()
    id_d = nc.inline_tensor(np.eye(128, dtype=np.float32), "ident").ap()

    # [t, d] views tiled as [chunk, partition, j, d] with 4x128 rows per chunk
    u_t = u_d.rearrange("(c j p) d -> c p j d", p=128, j=4)
    y_t = y_d.rearrange("(c j p) d -> c p j d", p=128, j=4)

    with tile.TileContext(nc) as tc:
        with (
            tc.tile_pool(name="const", bufs=1) as constp,
            tc.tile_pool(name="upool", bufs=2 if with_du else 3) as upool,
            tc.tile_pool(name="ystg", bufs=2) as ystgp,
            tc.tile_pool(name="utp", bufs=2) as utp,
            tc.tile_pool(name="big", bufs=1) as big,
            tc.tile_pool(name="small", bufs=1) as small,
            tc.tile_pool(name="trps", bufs=2, space="PSUM") as trps,
            tc.tile_pool(name="bups", bufs=2, space="PSUM") as bups,
            tc.tile_pool(name="yps", bufs=4, space="PSUM") as yps,
            tc.tile_pool(name="dram", bufs=1, space="DRAM") as dramp,
        ):
            pools = (constp, upool, ystgp, utp, big, small, trps, bups, yps, dramp)

            # early params: only what phase A needs right away (the u-chunk
            # prefetches emitted by the first body get DMA priority over the
            # rest of the parameters)
            id_sb = constp.tile([128, 128], F32)
            nc.sync.dma_start(id_sb[:], id_d)
            bt_sb = constp.tile([128, KD, SD], F32R)  # B_w^T  [d, n]
            nc.sync.dma_start(
                bt_sb[:], bt_d.rearrange("(k p) n -> p k n", p=128)
            )

            late = {}

            def emit_late_params():
                if late:
                    return
                ct_sb = constp.tile([128, NSC, DM], F32R)  # C_w^T  [n, d]
                nc.sync.dma_start(
                    ct_sb[:], ct_d.rearrange("(c p) d -> p c d", p=128)
                )
                fl_sb = constp.tile([128, 1], F32)
                nc.sync.dma_start(fl_sb[:], fl_d)
                ll_sb = small.tile([128, NSC], F32)
                nc.sync.dma_start(
                    ll_sb[:], ll_d.rearrange("(c p) -> p c", p=128)
                )
                lam_sb = small.tile([128, NSC], F32)
                nc.scalar.activation(
                    lam_sb[:], ll_sb[:], mybir.ActivationFunctionType.Sigmoid
                )
                lam512 = constp.tile([128, NSC, 512], F32)
                nc.vector.memset(lam512[:], 1.0)
                for c in range(NSC):
                    nc.vector.tensor_scalar(
                        lam512[:, c],
                        lam512[:, c],
                        lam_sb[:, c : c + 1],
                        None,
                        mybir.AluOpType.mult,
                    )
                if with_du:
                    db_sb = constp.tile([128, DM], F32)
                    nc.sync.dma_start(db_sb[:], db_d)
                else:
                    db_sb = None
                late.update(ct_sb=ct_sb, fl_sb=fl_sb, lam512=lam512,
                            db_sb=db_sb)

            for _it in range(loop_n):
                _emit_body(nc, pools, id_sb, bt_sb, late, emit_late_params,
                           u_t, y_t, with_du)

    nc.compile()
    return nc


def _emit_body(nc, pools, id_sb, bt_sb, late, emit_late_params, u_t, y_t,
               with_du):
    constp, upool, ystgp, utp, big, small, trps, bups, yps, dramp = pools

    # with du: one resident staging buffer holds D*u until phase C adds to
    # it. without du: a small rotating staging pool frees SBUF for a third
    # u buffer (keeps the input DMA stream gapless).
    y_acc = big.tile([128, 4 * NTC, DM], F32) if with_du else None
    but = big.tile([128, NSC, TH], F32)  # Bu^T   [n, t]
    hs = big.tile([128, NSC, TH], F32)  # h^T    [n, t]

    # ---- phase A: load u, transpose, B-projection ----
    for tc_i in range(NTC):
        u_sb = upool.tile([128, 4, DM], F32)
        nc.sync.dma_start(u_sb[:], u_t[tc_i])
        if tc_i == 0:
            emit_late_params()
        ct_sb = late["ct_sb"]
        fl_sb = late["fl_sb"]
        lam512 = late["lam512"]

        if with_du:
            # D * u while u is resident in natural layout
            for j in range(4):
                nc.gpsimd.tensor_tensor(
                    y_acc[:, 4 * tc_i + j],
                    u_sb[:, j],
                    late["db_sb"][:],
                    mybir.AluOpType.mult,
                )  # noqa: du path keeps the resident y_acc

        ut_sb = utp.tile([128, KD, 512], F32)  # u^T [d, t] chunk
        for k in range(KD):
            tp = trps.tile([128, 512], F32, tag="trp")
            for j in range(4):
                nc.tensor.transpose(
                    tp[:, 128 * j : 128 * (j + 1)],
                    u_sb[:, j, 128 * k : 128 * (k + 1)],
                    id_sb[:],
                )
            # ACT rounds to fp32r on writeback, as the fp32r matmul requires
            nc.scalar.copy(_r(ut_sb[:, k]), tp[:])

        for n in range(NSC):
            bp = bups.tile([128, 512], F32, tag="bup")
            for k in range(KD):
                nc.tensor.matmul(
                    bp[:],
                    bt_sb[:, k, 128 * n : 128 * (n + 1)],
                    _r(ut_sb[:, k]),
                    start=(k == 0),
                    stop=(k == KD - 1),
                )
            nc.scalar.copy(but[:, n, 512 * tc_i : 512 * (tc_i + 1)], bp[:])

    # ---- phase B: local scan, boundary exchange, final scan ----
    for c in range(NSC):
        for s in range(NTC):
            init = 0.0 if s == 0 else hs[:, c, 512 * s - 1 : 512 * s]
            nc.vector.tensor_tensor_scan(
                _r(hs[:, c, 512 * s : 512 * (s + 1)]),
                lam512[:, c],
                but[:, c, 512 * s : 512 * (s + 1)],
                init,
                mybir.AluOpType.mult,
                mybir.AluOpType.add,
            )

    f_sb = small.tile([128, NSC], F32)
    for c in range(NSC):
        nc.vector.tensor_copy(f_sb[:, c : c + 1], hs[:, c, TH - 1 : TH])
    f_dram = dramp.tile([NSC, 128], F32)
    fg_dram = dramp.tile([2, NSC, 128], F32)
    nc.sync.dma_start(f_dram.rearrange("c p -> p c"), f_sb[:])
    nc.gpsimd.collective_compute(
        "AllGather",
        mybir.AluOpType.bypass,
        replica_groups=GROUPS,
        ins=[f_dram.opt()],
        outs=[fg_dram.opt()],
    )
    finit = small.tile([128, NSC], F32)
    # rank 0 of each pair = first-half core: its final state
    nc.sync.dma_start(finit[:], fg_dram[0].rearrange("c p -> p c"))
    nc.vector.tensor_scalar(
        finit[:], finit[:], fl_sb[:, 0:1], None, mybir.AluOpType.mult
    )

    for c in range(NSC):
        for s in range(NTC):
            init = (
                finit[:, c : c + 1]
                if s == 0
                else hs[:, c, 512 * s - 1 : 512 * s]
            )
            nc.vector.tensor_tensor_scan(
                _r(hs[:, c, 512 * s : 512 * (s + 1)]),
                lam512[:, c],
                but[:, c, 512 * s : 512 * (s + 1)],
                init,
                mybir.AluOpType.mult,
                mybir.AluOpType.add,
            )

    # ---- phase C: C-projection and output ----
    for tc_i in range(NTC):
        ystg = y_acc[:, 4 * tc_i : 4 * (tc_i + 1)] if with_du else \
            ystgp.tile([128, 4, DM], F32)
        for j in range(4):
            tt = 4 * tc_i + j
            yp = [
                yps.tile([128, 512], F32, tag="yp", name=f"yp{tt}_{i}")
                for i in range(2)
            ]
            for c in range(NSC):
                for dh in range(2):
                    nc.tensor.matmul(
                        yp[dh][:],
                        _r(hs[:, c, 128 * tt : 128 * (tt + 1)]),
                        ct_sb[:, c, 512 * dh : 512 * (dh + 1)],
                        start=(c == 0),
                        stop=(c == NSC - 1),
                    )
            for dh in range(2):
                sl = slice(512 * dh, 512 * (dh + 1))
                if with_du:
                    nc.vector.tensor_tensor(
                        ystg[:, j, sl], yp[dh][:], ystg[:, j, sl],
                        mybir.AluOpType.add,
                    )
                elif dh == 0:
                    nc.scalar.copy(ystg[:, j, sl], yp[dh][:])
                else:
                    nc.vector.tensor_copy(ystg[:, j, sl], yp[dh][:])
        nc.sync.dma_start(y_t[tc_i], ystg[:])


_NC_CACHE = {}
LAST_RESULT = None


def _get_program(with_du):
    if with_du not in _NC_CACHE:
        _NC_CACHE[with_du] = build_program(with_du=with_du)
    return _NC_CACHE[with_du]


def make_in_maps(u, log_lambda, B_w, C_w, D):
    u = np.ascontiguousarray(np.asarray(u, dtype=np.float32))
    log_lambda = np.asarray(log_lambda, dtype=np.float32)
    bt = np.ascontiguousarray(np.asarray(B_w, dtype=np.float32).T)
    ct = np.ascontiguousarray(np.asarray(C_w, dtype=np.float32).T)
    dbc = np.ascontiguousarray(
        np.broadcast_to(np.asarray(D, dtype=np.float32), (128, DM))
    )
    in_maps = []
    for core in range(NCORES):
        b, h = core // 2, core % 2
        in_maps.append(
            {
                "u": np.ascontiguousarray(u[b, h * TH : (h + 1) * TH]),
                "logl": log_lambda,
                "bt": bt,
                "ct": ct,
                "dbc": dbc,
                "flag": np.full((128, 1), float(h), dtype=np.float32),
            }
        )
    return in_maps


def kernel(u, log_lambda, B_w, C_w, D):
    global LAST_RESULT
    with_du = bool(np.any(np.asarray(D)))
    nc = _get_program(with_du)
    in_maps = make_in_maps(u, log_lambda, B_w, C_w, D)
    try:
        res = run_bass_kernel_spmd(nc, in_maps, list(range(NCORES)))
    except Exception:
        # one retry: a prior crashed session can leave the device wedged
        # transiently; a fresh NRT session usually recovers it
        res = run_bass_kernel_spmd(nc, in_maps, list(range(NCORES)))
    LAST_RESULT = res
    y = np.empty((BATCH, SEQ, DM), dtype=np.float32)
    for core in range(NCORES):
        b, h = core // 2, core % 2
        y[b, h * TH : (h + 1) * TH] = res.results[core]["y"]
    return y


# revision 19
# speedup vs baseline: 3.2057x; 3.2057x over previous
"""Trainium2 Bass kernel for a diagonal SSM layer.

Computes, for u [4, 4096, 1024]:
    lam = sigmoid(log_lambda)                 # [256]
    Bu  = einsum('bsd,nd->bsn', u, B_w)       # [4, 4096, 256]
    h_t = lam * h_{t-1} + Bu_t                # scan over s
    y   = einsum('bsn,dn->bsd', hs, C_w) + D * u

Sharding: 8 cores = 4 batches x 2 sequence halves (2048 steps each).
Parameters are replicated. The half-boundary state is exchanged between
core pairs (2b, 2b+1) with a tiny AllGather; each core then re-runs its
(cheap) scan seeded with the inherited state. Uniform SPMD program —
even cores multiply the inherited state by a per-core flag input of 0.

Per-core dataflow:
  DMA u -> PE transpose (128x128 blocks, via identity matmul) -> uT
  BuT[n,t] = (B_w^T)^T @ uT          (float32r matmuls, K=1024)
  scan1 over t (DVE tensor_tensor_scan, state dim on partitions)
  exchange final state within the core pair; scan2 with inherited init
  y[t,:]  = hsT^T @ C_w^T (+ D*u)    (float32r matmuls, K=256)
  DMA y out

float32r is tf32 on the PE (10-bit mantissa, full-rate); everything
outside the matmuls stays fp32, overall relative error ~2e-4.

Two program variants are built: the D==0 fast path skips the D*u
elementwise stream (D is exactly zero for this layer's initialization);
any nonzero D dispatches to the general variant at call time.
"""

import sys

import numpy as np

sys.path.insert(0, "/opt/trn_rl_repo")

from concourse import bacc, mybir  # noqa: E402
import concourse.tile as tile  # noqa: E402
from concourse.bass_utils import run_bass_kernel_spmd  # noqa: E402

BATCH, SEQ, DM, SD = 4, 4096, 1024, 256
NCORES = 8
TH = SEQ // 2  # timesteps per core
NTC = TH // 512  # 512-step chunks per core
KD = DM // 128  # contraction chunks for the B matmul
NSC = SD // 128  # state chunks

F32 = mybir.dt.float32
F32R = mybir.dt.float32r

GROUPS = [[0, 1], [2, 3], [4, 5], [6, 7]]


def _r(ap):
    """float32r (tf32) view: full-rate fp32 matmul on the PE array."""
    return ap.bitcast(F32R)


def _f(ap):
    """plain-fp32 view of a float32r tile (bitwise identical)."""
    return ap.bitcast(F32)


def build_program(loop_n=1, with_du=False, num_devices=NCORES):
    nc = bacc.Bacc(
        "TRN2", target_bir_lowering=False, debug=False, num_devices=num_devices
    )

    u_d = nc.dram_tensor("u", [TH, DM], F32, kind="ExternalInput").ap()
    ll_d = nc.dram_tensor("logl", [SD], F32, kind="ExternalInput").ap()
    bt_d = nc.dram_tensor("bt", [DM, SD], F32R, kind="ExternalInput").ap()
    ct_d = nc.dram_tensor("ct", [SD, DM], F32R, kind="ExternalInput").ap()
    db_d = nc.dram_tensor("dbc", [128, DM], F32, kind="ExternalInput").ap()
    fl_d = nc.dram_tensor("flag", [128, 1], F32, kind="ExternalInput").ap()
    y_d = nc.dram_tensor("y", [TH, DM], F32, kind="ExternalOutput").ap()
    id_d = nc.inline_tensor(np.eye(128, dtype=np.float32), "ident").ap()

    # [t, d] views tiled as [chunk, partition, j, d]; u in 4x128-row
    # chunks, y in 2x128-row half-chunks (earlier first output DMA)
    u_t = u_d.rearrange("(c j p) d -> c p j d", p=128, j=4)
    y_t = y_d.rearrange("(h j p) d -> h p j d", p=128, j=2)

    with tile.TileContext(nc) as tc:
        with (
            tc.tile_pool(name="const", bufs=1) as constp,
            tc.tile_pool(name="upool", bufs=2 if with_du else 3) as upool,
            tc.tile_pool(name="ystg", bufs=3) as ystgp,
            tc.tile_pool(name="utp", bufs=2) as utp,
            tc.tile_pool(name="big", bufs=1) as big,
            tc.tile_pool(name="small", bufs=1) as small,
            tc.tile_pool(name="trps", bufs=2, space="PSUM") as trps,
            tc.tile_pool(name="bups", bufs=2, space="PSUM") as bups,
            tc.tile_pool(name="yps", bufs=4, space="PSUM") as yps,
            tc.tile_pool(name="dram", bufs=1, space="DRAM") as dramp,
        ):
            pools = (constp, upool, ystgp, utp, big, small, trps, bups, yps, dramp)

            # early params: only what phase A needs right away (the u-chunk
            # prefetches emitted by the first body get DMA priority over the
            # rest of the parameters)
            id_sb = constp.tile([128, 128], F32)
            nc.sync.dma_start(id_sb[:], id_d)
            bt_sb = constp.tile([128, KD, SD], F32R)  # B_w^T  [d, n]
            nc.sync.dma_start(
                bt_sb[:], bt_d.rearrange("(k p) n -> p k n", p=128)
            )

            late = {}

            def emit_late_params():
                if late:
                    return
                ct_sb = constp.tile([128, NSC, DM], F32R)  # C_w^T  [n, d]
                nc.sync.dma_start(
                    ct_sb[:], ct_d.rearrange("(c p) d -> p c d", p=128)
                )
                fl_sb = constp.tile([128, 1], F32)
                nc.sync.dma_start(fl_sb[:], fl_d)
                ll_sb = small.tile([128, NSC], F32)
                nc.sync.dma_start(
                    ll_sb[:], ll_d.rearrange("(c p) -> p c", p=128)
                )
                lam_sb = small.tile([128, NSC], F32)
                nc.scalar.activation(
                    lam_sb[:], ll_sb[:], mybir.ActivationFunctionType.Sigmoid
                )
                lam512 = constp.tile([128, NSC, 512], F32)
                nc.vector.memset(lam512[:], 1.0)
                for c in range(NSC):
                    nc.vector.tensor_scalar(
                        lam512[:, c],
                        lam512[:, c],
                        lam_sb[:, c : c + 1],
                        None,
                        mybir.AluOpType.mult,
                    )
                if with_du:
                    db_sb = constp.tile([128, DM], F32)
                    nc.sync.dma_start(db_sb[:], db_d)
                else:
                    db_sb = None
                late.update(ct_sb=ct_sb, fl_sb=fl_sb, lam512=lam512,
                            db_sb=db_sb)

            for _it in range(loop_n):
                _emit_body(nc, pools, id_sb, bt_sb, late, emit_late_params,
                           u_t, y_t, with_du)

    nc.compile()
    return nc


def _emit_body(nc, pools, id_sb, bt_sb, late, emit_late_params, u_t, y_t,
               with_du):
    constp, upool, ystgp, utp, big, small, trps, bups, yps, dramp = pools

    # with du: one resident staging buffer holds D*u until phase C adds to
    # it. without du: a small rotating staging pool frees SBUF for a third
    # u buffer (keeps the input DMA stream gapless).
    y_acc = big.tile([128, 4 * NTC, DM], F32) if with_du else None
    but = big.tile([128, NSC, TH], F32)  # Bu^T   [n, t]
    hs = big.tile([128, NSC, TH], F32)  # h^T    [n, t]

    # ---- phase A: load u, transpose, B-projection ----
    for tc_i in range(NTC):
        u_sb = upool.tile([128, 4, DM], F32)
        nc.sync.dma_start(u_sb[:], u_t[tc_i])
        if tc_i == 0:
            emit_late_params()
        ct_sb = late["ct_sb"]
        fl_sb = late["fl_sb"]
        lam512 = late["lam512"]

        if with_du:
            # D * u while u is resident in natural layout
            for j in range(4):
                nc.gpsimd.tensor_tensor(
                    y_acc[:, 4 * tc_i + j],
                    u_sb[:, j],
                    late["db_sb"][:],
                    mybir.AluOpType.mult,
                )  # noqa: du path keeps the resident y_acc

        ut_sb = utp.tile([128, KD, 512], F32)  # u^T [d, t] chunk
        for k in range(KD):
            tp = trps.tile([128, 512], F32, tag="trp")
            for j in range(4):
                nc.tensor.transpose(
                    tp[:, 128 * j : 128 * (j + 1)],
                    u_sb[:, j, 128 * k : 128 * (k + 1)],
                    id_sb[:],
                )
            # ACT rounds to fp32r on writeback, as the fp32r matmul requires
            nc.scalar.copy(_r(ut_sb[:, k]), tp[:])

        for n in range(NSC):
            bp = bups.tile([128, 512], F32, tag="bup")
            for k in range(KD):
                nc.tensor.matmul(
                    bp[:],
                    bt_sb[:, k, 128 * n : 128 * (n + 1)],
                    _r(ut_sb[:, k]),
                    start=(k == 0),
                    stop=(k == KD - 1),
                )
            nc.scalar.copy(but[:, n, 512 * tc_i : 512 * (tc_i + 1)], bp[:])

    # ---- phase B: local scan, boundary exchange, final scan ----
    for c in range(NSC):
        for s in range(NTC):
            init = 0.0 if s == 0 else hs[:, c, 512 * s - 1 : 512 * s]
            nc.vector.tensor_tensor_scan(
                _r(hs[:, c, 512 * s : 512 * (s + 1)]),
                lam512[:, c],
                but[:, c, 512 * s : 512 * (s + 1)],
                init,
                mybir.AluOpType.mult,
                mybir.AluOpType.add,
            )

    f_sb = small.tile([128, NSC], F32)
    for c in range(NSC):
        nc.vector.tensor_copy(f_sb[:, c : c + 1], hs[:, c, TH - 1 : TH])
    f_dram = dramp.tile([NSC, 128], F32)
    fg_dram = dramp.tile([2, NSC, 128], F32)
    nc.sync.dma_start(f_dram.rearrange("c p -> p c"), f_sb[:])
    nc.gpsimd.collective_compute(
        "AllGather",
        mybir.AluOpType.bypass,
        replica_groups=GROUPS,
        ins=[f_dram.opt()],
        outs=[fg_dram.opt()],
    )
    finit = small.tile([128, NSC], F32)
    # rank 0 of each pair = first-half core: its final state
    nc.sync.dma_start(finit[:], fg_dram[0].rearrange("c p -> p c"))
    nc.vector.tensor_scalar(
        finit[:], finit[:], fl_sb[:, 0:1], None, mybir.AluOpType.mult
    )

    for c in range(NSC):
        for s in range(NTC):
            init = (
                finit[:, c : c + 1]
                if s == 0
                else hs[:, c, 512 * s - 1 : 512 * s]
            )
            nc.vector.tensor_tensor_scan(
                _r(hs[:, c, 512 * s : 512 * (s + 1)]),
                lam512[:, c],
                but[:, c, 512 * s : 512 * (s + 1)],
                init,
                mybir.AluOpType.mult,
                mybir.AluOpType.add,
            )

    # ---- phase C: C-projection and output ----
    # half-chunk (2-tile / 1 MB) output granularity: the first y DMA
    # launches after only two matmul+evac rounds, smoothing the tail
    for h_i in range(2 * NTC):
        ystg = (
            y_acc[:, 2 * h_i : 2 * (h_i + 1)]
            if with_du
            else ystgp.tile([128, 2, DM], F32)
        )
        for j in range(2):
            tt = 2 * h_i + j
            yp = [
                yps.tile([128, 512], F32, tag="yp", name=f"yp{tt}_{i}")
                for i in range(2)
            ]
            for c in range(NSC):
                for dh in range(2):
                    nc.tensor.matmul(
                        yp[dh][:],
                        _r(hs[:, c, 128 * tt : 128 * (tt + 1)]),
                        ct_sb[:, c, 512 * dh : 512 * (dh + 1)],
                        start=(c == 0),
                        stop=(c == NSC - 1),
                    )
            for dh in range(2):
                sl = slice(512 * dh, 512 * (dh + 1))
                if with_du:
                    nc.vector.tensor_tensor(
                        ystg[:, j, sl], yp[dh][:], ystg[:, j, sl],
                        mybir.AluOpType.add,
                    )
                elif dh == 0:
                    nc.scalar.copy(ystg[:, j, sl], yp[dh][:])
                else:
                    nc.vector.tensor_copy(ystg[:, j, sl], yp[dh][:])
        nc.sync.dma_start(y_t[h_i], ystg[:])


_NC_CACHE = {}
LAST_RESULT = None


def _get_program(with_du):
    if with_du not in _NC_CACHE:
        _NC_CACHE[with_du] = build_program(with_du=with_du)
    return _NC_CACHE[with_du]


def make_in_maps(u, log_lambda, B_w, C_w, D):
    u = np.ascontiguousarray(np.asarray(u, dtype=np.float32))
    log_lambda = np.asarray(log_lambda, dtype=np.float32)
    bt = np.ascontiguousarray(np.asarray(B_w, dtype=np.float32).T)
    ct = np.ascontiguousarray(np.asarray(C_w, dtype=np.float32).T)
    dbc = np.ascontiguousarray(
        np.broadcast_to(np.asarray(D, dtype=np.float32), (128, DM))
    )
    in_maps = []
    for core in range(NCORES):
        b, h = core // 2, core % 2
        in_maps.append(
            {
                "u": np.ascontiguousarray(u[b, h * TH : (h + 1) * TH]),
                "logl": log_lambda,
                "bt": bt,
                "ct": ct,
                "dbc": dbc,
                "flag": np.full((128, 1), float(h), dtype=np.float32),
            }
        )
    return in_maps


def kernel(u, log_lambda, B_w, C_w, D):
    global LAST_RESULT
    with_du = bool(np.any(np.asarray(D)))
    nc = _get_program(with_du)
    in_maps = make_in_maps(u, log_lambda, B_w, C_w, D)
    try:
        res = run_bass_kernel_spmd(nc, in_maps, list(range(NCORES)))
    except Exception:
        # one retry: a prior crashed session can leave the device wedged
        # transiently; a fresh NRT session usually recovers it
        res = run_bass_kernel_spmd(nc, in_maps, list(range(NCORES)))
    LAST_RESULT = res
    y = np.empty((BATCH, SEQ, DM), dtype=np.float32)
    for core in range(NCORES):
        b, h = core // 2, core % 2
        y[b, h * TH : (h + 1) * TH] = res.results[core]["y"]
    return y
